# revision 1
# baseline (speedup 1.0000x reference)
"""Sliding-window multi-head attention (window +-64, S=2048, H=8, hd=64)
for 8 Trainium2 NeuronCores.

Sharding: sequence-parallel. Core c owns queries [c*256, (c+1)*256); it
receives x^T columns for its query range plus a 64-column halo on each side
(zero padded at the sequence edges), computes Q/K/V projections locally
(weights replicated), runs banded softmax-attention for all 8 heads, applies
the output projection, and writes its y^T block. The host reassembles
y = concat_c(yT_c.T) and adds the (input-dependent) constant bias
b_eff = b_o + w_o @ b_v, which is exact because softmax rows sum to 1.

Per (head, 128-query tile) the score span is the 256 keys [i-64, i+192);
the band mask is applied by accumulating identity @ mask_additive into the
scores PSUM on the tensor engine, so exp (+row-sum accumulator) can run
straight out of PSUM on the scalar engine. Attention rows are transposed
on PE (128x128 blocks) onto an absolute 3x128-key grid for the AV matmuls;
the two never-written corner blocks of that grid are pre-zeroed once.

Self-contained: hardcodes all shapes; no sibling imports.
"""

import numpy as np

import concourse.bass as bass
import concourse.tile as tile
from concourse import bacc, mybir
from concourse.bass_utils import run_bass_kernel_spmd

# problem shapes
S = 2048          # sequence length
E = 512           # embed dim (= d_in)
H = 8             # heads
HD = E // H       # head dim, 64
HWIN = 64         # half window (attend to |q-k| <= 64)
N_CORES = 8
SLOC = S // N_CORES       # queries per core, 256
HALO = SLOC + 2 * HWIN    # local x/k/v span, 384
NT = SLOC // 128          # q tiles per core, 2
KC = HALO // 128          # key chunks per core, 3
SPAN = 256                # keys per q tile: [i-64, i+192)
P = 128

F32 = mybir.dt.float32
F32R = mybir.dt.float32r

# knobs
MM_DTYPE = F32R      # dtype for matmul operands (F32 = exact 4c/row, F32R fast)
MASK_MODE = "mm"     # "mm": identity@mask on PE; "stt": mask+sum on DVE
NORM_ENGINE = "pool"  # engine for att = p * (1/sum): dve | act | pool
NEG = -1e30          # additive mask value
ATT_BF16 = False     # att matrix + V in bf16: 1.4us faster, 8x the error
FEW_DMAS = True      # one DMA per weight matrix (7 total) vs chunked (23)
WARMUP_MMS = 10      # dummy matmuls during the DMA ramp to lift the PE HAM

MMD = MM_DTYPE
BF16 = mybir.dt.bfloat16
ATT_DT = BF16 if ATT_BF16 else MM_DTYPE


def _build_kernel(nc: bass.Bass, reps: int = 1):
    """Emit the SPMD per-core program. All per-core variation comes from the
    input tensors. reps>1 repeats the body inside one NEFF (benchmarking)."""
    act_f = mybir.ActivationFunctionType

    # ---- I/O ----
    xT = nc.dram_tensor("xT", [E, HALO], MMD, kind="ExternalInput").ap()
    wqT = nc.dram_tensor("wqT", [E, E], MMD, kind="ExternalInput").ap()
    wkT = nc.dram_tensor("wkT", [E, E], MMD, kind="ExternalInput").ap()
    wvT = nc.dram_tensor("wvT", [E, E], MMD, kind="ExternalInput").ap()
    woT = nc.dram_tensor("woT", [E, E], MMD, kind="ExternalInput").ap()
    # packed constants:
    # [ident(128) | mask0(256) | mask1(256) | bq(4) | bk(4) | ident_bf16(64)]
    CW = P + NT * SPAN + 8 + (64 if ATT_BF16 else 0)
    cst = nc.dram_tensor("cst", [P, CW], MMD, kind="ExternalInput").ap()
    yT = nc.dram_tensor("yT", [E, SLOC], F32, kind="ExternalOutput").ap()

    with tile.TileContext(nc) as tc:
        with (
            tc.tile_pool(name="consts", bufs=1) as consts,
            tc.tile_pool(name="persist", bufs=1) as persist,
            tc.tile_pool(name="work", bufs=10) as work,
            tc.tile_pool(name="ps_qkv", bufs=2, space="PSUM") as ps_qkv,
            tc.tile_pool(name="ps_pt", bufs=2, space="PSUM") as ps_pt,
            tc.tile_pool(name="ps_av", bufs=2, space="PSUM") as ps_av,
            tc.tile_pool(name="ps_y", bufs=1, space="PSUM") as ps_y,
        ):
            def emit():
                # warm the PE clock gate during the load ramp: dummy
                # matmuls on a zeroed scratch tile, no data dependencies
                if WARMUP_MMS:
                    wsc = work.tile([P, E], MMD, tag="warm", name="warm")
                    nc.gpsimd.memset(wsc[:].bitcast(F32), 0.0)
                    wps = ps_qkv.tile([P, E], F32, tag="qkv", name="qkv")
                    for _ in range(WARMUP_MMS):
                        nc.tensor.matmul(wps[:], wsc[:, 0:P], wsc[:],
                                         start=True, stop=True)
                # ---- loads, in consumption order ----
                def load_w(name, ap):
                    out = []
                    for k in range(4):
                        w = persist.tile([P, E], MMD, tag=f"{name}{k}",
                                         name=f"{name}{k}")
                        nc.sync.dma_start(w[:], ap[k * P:(k + 1) * P, :])
                        out.append(w)
                    return out

                cst_sb = consts.tile([P, CW], MMD, tag="cst", name="cst")

                def w_col_tile(name, c, ap4):
                    w = persist.tile([P, 4, P], MMD, tag=f"{name}{c}",
                                     name=f"{name}{c}")
                    nc.sync.dma_start(w[:], ap4[:, :, c * P:(c + 1) * P])
                    return w

                wq4 = wqT.rearrange("(k p) c -> p k c", p=P)
                wk4 = wkT.rearrange("(k p) c -> p k c", p=P)
                wq_sb, wk_sb = [], []
                x_sb = persist.tile([P, 4, HALO], MMD, tag="x", name="x")
                xT4 = xT.rearrange("(k p) s -> p k s", p=P)
                if FEW_DMAS:
                    nc.sync.dma_start(x_sb[:, 0:2, :], xT4[:, 0:2, :])
                    nc.sync.dma_start(x_sb[:, 2:4, :], xT4[:, 2:4, :])
                else:
                    nc.sync.dma_start(x_sb[:, 0:2, :], xT4[:, 0:2, :])
                    nc.sync.dma_start(x_sb[:, 2:4, :], xT4[:, 2:4, :])
                if FEW_DMAS:
                    def w_half_tile(name, ap):
                        w = persist.tile([P, 4, E], MMD, tag=name, name=name)
                        return w, ap.rearrange("(k p) c -> p k c", p=P)

                    wq_t, wq_ap = w_half_tile("wq", wqT)
                    wk_t, wk_ap = w_half_tile("wk", wkT)
                    nc.sync.dma_start(wq_t[:, :, 0:P], wq_ap[:, :, 0:P])
                    nc.sync.dma_start(wk_t[:, :, 0:P], wk_ap[:, :, 0:P])
                    wq_sb = [wq_t[:, :, c * P:(c + 1) * P] for c in range(4)]
                    wk_sb = [wk_t[:, :, c * P:(c + 1) * P] for c in range(4)]
                else:
                    wq_sb.append(w_col_tile("wq", 0, wq4))
                    wk_sb.append(w_col_tile("wk", 0, wk4))
                nc.sync.dma_start(cst_sb[:, 0:P + SPAN], cst[:, 0:P + SPAN])
                nc.sync.dma_start(cst_sb[:, P + NT * SPAN:],
                                  cst[:, P + NT * SPAN:])
                if FEW_DMAS:
                    nc.sync.dma_start(cst_sb[:, P + SPAN:P + NT * SPAN],
                                      cst[:, P + SPAN:P + NT * SPAN])
                    nc.sync.dma_start(wq_t[:, :, P:2 * P], wq_ap[:, :, P:2 * P])
                    nc.sync.dma_start(wk_t[:, :, P:2 * P], wk_ap[:, :, P:2 * P])
                ident_sb = cst_sb[:, 0:P]
                mask_sb = [cst_sb[:, P + t * SPAN:P + (t + 1) * SPAN]
                           for t in range(NT)]
                bq_sb = cst_sb[:, P + NT * SPAN:P + NT * SPAN + 4].bitcast(F32)
                bk_sb = cst_sb[:, P + NT * SPAN + 4:
                               P + NT * SPAN + 8].bitcast(F32)
                if ATT_BF16:
                    ident_att = cst_sb[:, P + NT * SPAN + 8:].bitcast(BF16)
                else:
                    ident_att = cst_sb[:, 0:P]
                if not FEW_DMAS:
                    wq_sb.append(w_col_tile("wq", 1, wq4))
                    wk_sb.append(w_col_tile("wk", 1, wk4))
                if FEW_DMAS:
                    def load_w_rows(name, ap):
                        w = persist.tile([P, 4, E], MMD, tag=name, name=name)
                        nc.sync.dma_start(
                            w[:], ap.rearrange("(k p) c -> p k c", p=P))
                        return [w[:, k, :] for k in range(4)]
                    wv_sb = load_w_rows("wv", wvT)
                    nc.sync.dma_start(wq_t[:, :, 2 * P:], wq_ap[:, :, 2 * P:])
                    nc.sync.dma_start(wk_t[:, :, 2 * P:], wk_ap[:, :, 2 * P:])
                else:
                    wv_sb = load_w("wv", wvT)
                if not FEW_DMAS:
                    for c in range(2, 4):
                        wq_sb.append(w_col_tile("wq", c, wq4))
                        wk_sb.append(w_col_tile("wk", c, wk4))
                if FEW_DMAS:
                    wo_sb = load_w_rows("wo", woT)
                else:
                    wo_sb = load_w("wo", woT)

                # ---- QKV projections (emitted per-chunk, interleaved
                # with attention PASS A below) ----
                qT_sb, kT_sb = [None] * 4, [None] * 4

                def emit_qk_chunk(c):
                    for nm, dst, w_sb, b_sb, cols in (
                        ("q", qT_sb, wq_sb, bq_sb, SLOC),
                        ("k", kT_sb, wk_sb, bk_sb, HALO),
                    ):
                        x_off = HWIN if cols == SLOC else 0
                        ps = ps_qkv.tile([P, cols], F32, tag="qkv", name="qkv")
                        for k in range(4):
                            nc.tensor.matmul(
                                ps[:], w_sb[c][:, k, :],
                                x_sb[:, k, x_off:x_off + cols],
                                start=(k == 0), stop=(k == 3),
                            )
                        sb = persist.tile([P, cols], MMD, tag=f"{nm}T{c}",
                                          name=f"{nm}T{c}")
                        nc.vector.tensor_scalar_add(
                            sb[:], ps[:], b_sb[:, c:c + 1])
                        dst[c] = sb
                # ---- attention ----
                # absolute-grid transposed attention, one buffer per head
                # parity; layout [p, (c t) * 128] c<3, t<2; corner blocks
                # (c0,t1)=idx1 and (c2,t0)=idx4 stay zero.
                attbuf = []
                for par in range(H):
                    ab = persist.tile([P, KC * NT * P], ATT_DT,
                                      tag=f"attT{par}", name=f"attT{par}")
                    ab3 = ab[:].rearrange("p (b q) -> p b q", q=P)
                    nc.gpsimd.memset(ab3[:, 1:5:3, :].bitcast(F32), 0.0)
                    attbuf.append(ab)

                valsT_sb = [
                    persist.tile([P, SLOC], MMD, tag=f"valsT{c}",
                                 name=f"valsT{c}")
                    for c in range(4)
                ]
                # yT accumulators: o-chunks packed in pairs per PSUM bank
                y_ps = [ps_y.tile([P, SLOC], F32, tag=f"y{i}",
                                  name=f"y{i}") for i in range(2)]

                def emit_y_accum(f):
                    for o in range(2):
                        nc.tensor.matmul(
                            y_ps[o][:],
                            wo_sb[f][:, o * P:(o + 1) * P],
                            valsT_sb[f][:], start=(f == 0), stop=(f == 3),
                        )

                scale = 1.0 / float(np.sqrt(HD))

                def emit_v_proj():
                    v_sb = []
                    for skc in range(KC):
                        ps = ps_qkv.tile([P, E], F32, tag="qkv", name="qkv")
                        for k in range(4):
                            nc.tensor.matmul(
                                ps[:], x_sb[:, k, skc * P:(skc + 1) * P],
                                wv_sb[k][:] if not FEW_DMAS else wv_sb[k],
                                start=(k == 0), stop=(k == 3),
                            )
                        sb = persist.tile([P, E], ATT_DT, tag=f"v{skc}",
                                          name=f"v{skc}")
                        nc.vector.tensor_copy(sb[:], ps[:])
                        v_sb.append(sb)
                    return v_sb

                def emit_pass_b(h):
                    c, r = h // 2, (h % 2) * HD
                    ab = attbuf[h]
                    av = ps_av.tile([HD, SLOC], F32, tag="av", name="av")
                    for kc in range(KC):
                        nc.tensor.matmul(
                            av[:],
                            v_sb[kc][:, h * HD:(h + 1) * HD],
                            ab[:, kc * SLOC:(kc + 1) * SLOC],
                            start=(kc == 0), stop=(kc == KC - 1),
                        )
                    if h % 2 == 0:
                        nc.vector.tensor_copy(valsT_sb[c][r:r + HD, :], av[:])
                    else:
                        nc.scalar.copy(valsT_sb[c][r:r + HD, :], av[:])
                    if h % 2 == 1:
                        emit_y_accum(c)

                # PASS A: per (head, tile) scores -> masked exp -> normalize
                # -> transpose into the head's absolute-grid buffer;
                # interleaved with QKV chunk emission (so no engine queue
                # blocks on late weight DMAs) and with PASS B of head h-4
                # (so the PE queue finishes AVs right behind the last
                # scores instead of after them all).
                v_sb = None
                for h in range(H):
                    c, r = h // 2, (h % 2) * HD
                    if h % 2 == 0:
                        emit_qk_chunk(c)
                    ab = attbuf[h]
                    ab3 = ab[:].rearrange("p (b q) -> p b q", q=P)
                    if h == 2:
                        v_sb = emit_v_proj()
                    if h >= 3:
                        emit_pass_b(h - 3)
                    for t in range(NT):
                        scores = ps_qkv.tile([P, SPAN], F32, tag="qkv",
                                             name="sc")
                        if MASK_MODE == "mm":
                            # mask first: it only needs early-arriving
                            # consts, so the in-order PE starts the group
                            # before qT/kT land
                            nc.tensor.matmul(
                                scores[:], ident_sb, mask_sb[t],
                                start=True, stop=False,
                            )
                        nc.tensor.matmul(
                            scores[:],
                            qT_sb[c][r:r + HD, t * P:(t + 1) * P],
                            kT_sb[c][r:r + HD, t * P: t * P + SPAN],
                            start=(MASK_MODE != "mm"), stop=True,
                        )
                        p = work.tile([P, SPAN], F32, tag="p", name="p")
                        sums = work.tile([P, 1], F32, tag="sums", name="sums")
                        if MASK_MODE == "mm":
                            nc.scalar.activation(
                                p[:], scores[:], act_f.Exp, scale=scale,
                                accum_out=sums[:])
                            pmm = p
                        else:
                            nc.scalar.activation(
                                p[:], scores[:], act_f.Exp, scale=scale)
                            pmm = work.tile([P, SPAN], MMD, tag="pmm",
                                            name="pmm")
                            nc.vector.scalar_tensor_tensor(
                                pmm[:], p[:], 1.0, mask_sb[t],
                                op0=mybir.AluOpType.mult,
                                op1=mybir.AluOpType.mult,
                                accum_out=sums[:])
                        recip = work.tile([P, 1], F32, tag="recip",
                                          name="recip")
                        nc.vector.reciprocal(recip[:], sums[:])
                        att = work.tile([P, SPAN], ATT_DT, tag="att",
                                        name="att")
                        if NORM_ENGINE == "dve" or h >= 6:
                            nc.vector.tensor_scalar_mul(att[:], pmm[:],
                                                        recip[:])
                        elif NORM_ENGINE == "pool":
                            nc.gpsimd.tensor_scalar_mul(att[:], pmm[:],
                                                        recip[:])
                        else:
                            nc.scalar.activation(att[:], pmm[:], act_f.Copy,
                                                 scale=recip[:])
                        ptp = ps_pt.tile([P, SPAN], ATT_DT, tag="pt",
                                         name="pt")
                        id_ap = ident_att
                        for kc in range(2):
                            nc.tensor.transpose(
                                ptp[:, kc * P:(kc + 1) * P],
                                att[:, kc * P:(kc + 1) * P],
                                id_ap,
                            )
                        # blocks (c=t+kc, t) -> index 2*(t+kc)+t = 3t+2kc
                        dst = ab3[:, 3 * t: 3 * t + 3: 2, :]
                        if t == 0:
                            nc.vector.tensor_copy(dst, ptp[:].rearrange(
                                "p (b q) -> p b q", q=P))
                        else:
                            nc.scalar.copy(dst, ptp[:].rearrange(
                                "p (b q) -> p b q", q=P))

                for h in range(5, H):
                    emit_pass_b(h)

                # ---- output: tail chunks o=2,3 then copy + store ----
                tail_ps = []
                for o in (2, 3):
                    ps = ps_qkv.tile([P, SLOC], F32, tag="qkv", name="qkv")
                    for f in range(4):
                        nc.tensor.matmul(
                            ps[:], wo_sb[f][:, o * P:(o + 1) * P],
                            valsT_sb[f][:], start=(f == 0), stop=(f == 3),
                        )
                    tail_ps.append(ps)
                ysb = work.tile([P, 4 * SLOC], F32, tag="yt", name="yt")
                for o in range(4):
                    src_ap = y_ps[o][:] if o < 2 else tail_ps[o - 2][:]
                    dst_ap = ysb[:, o * SLOC:(o + 1) * SLOC]
                    if o % 2 == 0:
                        nc.scalar.copy(dst_ap, src_ap)
                    else:
                        nc.vector.tensor_copy(dst_ap, src_ap)
                y4 = yT.rearrange("(o p) q -> p o q", p=P)
                ysb3 = ysb[:].rearrange("p (o q) -> p o q", q=SLOC)
                nc.sync.dma_start(y4[:, 0:2, :], ysb3[:, 0:2, :])
                nc.sync.dma_start(y4[:, 2:4, :], ysb3[:, 2:4, :])

            for _rep in range(reps):
                emit()

    return nc


_prog_cache = {}


def _get_program(reps: int = 1):
    key = (MM_DTYPE, MASK_MODE, NORM_ENGINE, FEW_DMAS, ATT_BF16, WARMUP_MMS,
           reps)
    if key not in _prog_cache:
        nc = bacc.Bacc(
            "TRN2", target_bir_lowering=False, debug=False,
            num_devices=N_CORES,
        )
        _build_kernel(nc, reps=reps)
        nc.compile()
        _prog_cache[key] = nc
    return _prog_cache[key]


def _make_in_maps(x, w_qkv, b_qkv, w_o):
    x2 = np.ascontiguousarray(np.asarray(x, np.float32).reshape(S, E))
    w_qkv = np.asarray(w_qkv, np.float32)
    b_qkv = np.asarray(b_qkv, np.float32)
    w_o = np.asarray(w_o, np.float32)

    # w_qkv rows for head h: [h*3hd, h*3hd+hd) = q, +hd = k, +2hd = v
    idx_q = np.concatenate(
        [np.arange(h * 3 * HD, h * 3 * HD + HD) for h in range(H)])
    idx_k = idx_q + HD
    idx_v = idx_q + 2 * HD
    wqT = np.ascontiguousarray(w_qkv[idx_q].T)   # [in, (h,d)]
    wkT = np.ascontiguousarray(w_qkv[idx_k].T)
    wvT = np.ascontiguousarray(w_qkv[idx_v].T)
    woT = np.ascontiguousarray(w_o.T)            # [(h,d), out]
    bq = np.ascontiguousarray(b_qkv[idx_q].reshape(4, P).T)  # [p, chunk]
    bk = np.ascontiguousarray(b_qkv[idx_k].reshape(4, P).T)
    ident = np.eye(P, dtype=np.float32)

    xT = x2.T  # [E, S]
    in_maps = []
    for core in range(N_CORES):
        q0 = core * SLOC
        lo = q0 - HWIN
        xt = np.zeros((E, HALO), np.float32)
        slo, shi = max(lo, 0), min(q0 + SLOC + HWIN, S)
        xt[:, slo - lo: shi - lo] = xT[:, slo:shi]

        m = np.full((NT, P, SPAN), NEG, np.float32)
        for t in range(NT):
            # key position for span col j: q0 + t*128 - 64 + j
            kpos = q0 + t * P - HWIN + np.arange(SPAN)
            qpos = (q0 + t * P + np.arange(P))[:, None]
            valid = (np.abs(kpos[None, :] - qpos) <= HWIN) \
                & (kpos[None, :] >= 0) & (kpos[None, :] < S)
            m[t] = np.where(valid, 0.0, NEG)

        parts = [ident, m[0], m[1], bq, bk]
        if ATT_BF16:
            import ml_dtypes
            ibf = np.eye(P, dtype=ml_dtypes.bfloat16)
            parts.append(ibf.view(np.uint8).reshape(P, P * 2)
                         .view(np.float32))
        cst = np.concatenate(parts, axis=1).astype(np.float32)
        in_maps.append({
            "xT": np.ascontiguousarray(xt),
            "wqT": wqT, "wkT": wkT, "wvT": wvT, "woT": woT,
            "cst": np.ascontiguousarray(cst),
        })
    return in_maps


last_result = None  # BassKernelResults of the most recent run (for profiling)


def kernel(x, padding_mask, w_qkv, b_qkv, w_o, b_o, trace=False):
    global last_result
    b_qkv = np.asarray(b_qkv, np.float32)
    w_o = np.asarray(w_o, np.float32)
    b_o = np.asarray(b_o, np.float32)
    idx_v = np.concatenate(
        [np.arange(h * 3 * HD + 2 * HD, (h + 1) * 3 * HD) for h in range(H)])
    # rows of softmax sum to 1 (padding_mask is all ones per spec), so the
    # v/out biases commute to a constant output offset; b_qkv[idx_v] is in
    # (h,d) order, matching w_o's input order
    b_eff = b_o + w_o @ b_qkv[idx_v]

    nc = _get_program()
    in_maps = _make_in_maps(x, w_qkv, b_qkv, w_o)
    res = run_bass_kernel_spmd(
        nc, in_maps, core_ids=list(range(N_CORES)), trace=trace)
    last_result = res
    y = np.concatenate([r["yT"].T for r in res.results], axis=0)  # [S, E]
    y = y + b_eff[None, :]
    return y.reshape(1, S, E).astype(np.float32)



# revision 22
# speedup vs baseline: 1.2062x; 1.2062x over previous
"""Sliding-window multi-head attention (window +-64, S=2048, H=8, hd=64)
for 8 Trainium2 NeuronCores.

Sharding: sequence-parallel. Core c owns queries [c*256, (c+1)*256); it
receives x^T columns for its query range plus a 64-column halo on each side
(zero padded at the sequence edges), computes Q/K/V projections locally
(weights replicated), runs banded softmax-attention for all 8 heads, applies
the output projection, and writes its y^T block in bf16. The host
reassembles y = concat_c(yT_c.T) and adds the constant bias
b_eff = b_o + w_o @ b_v (exact because softmax rows sum to 1).

All matmul operands are bf16 (operand rounding only; PSUM accumulation is
f32), which halves HBM traffic vs f32 and runs 1 cycle/row on PE at any
free size. Inputs are packed host-side into three bf16 DRAM tensors sized
for >=512B contiguous DMA runs: [x | ident | masks | biases], [wq|wk]
column-interleaved so the first Q/K chunk can land early, and [wv|wo].

Per (head, 128-query tile) the score span is the 256 keys [i-64, i+192);
both tiles of a head live in one [128,512] PSUM bank. The band mask is an
additive -1e30 matrix accumulated into the scores PSUM via identity @ mask
on PE, so exp (+row-sum accumulator) runs straight out of PSUM on the
scalar engine; normalization runs on DVE (bf16 out = 2x rate). Attention
rows are transposed on PE into a [128, 512] PSUM pair whose block order
[t0kc0|t0kc1|t1kc1|t1kc2] is exactly the per-head buffer the AV matmuls
consume, so one copy lands it and the AV skips the never-written corner
blocks of the absolute key grid. The PE queue is software-pipelined one
head deep: transposes of head h-1 and AV/output-projection matmuls of head
h-3 are emitted around the scores of head h so the in-order PE never waits
on the exp/normalize round trip.

Self-contained: hardcodes all shapes; no sibling imports.
"""

import numpy as np

import concourse.bass as bass
import concourse.tile as tile
from concourse import bacc, mybir
from concourse.bass_utils import run_bass_kernel_spmd

# problem shapes
S = 2048          # sequence length
E = 512           # embed dim (= d_in)
H = 8             # heads
HD = E // H       # head dim, 64
HWIN = 64         # half window (attend to |q-k| <= 64)
N_CORES = 8
SLOC = S // N_CORES       # queries per core, 256
HALO = SLOC + 2 * HWIN    # local x/k/v span, 384
NT = SLOC // 128          # q tiles per core, 2
KC = HALO // 128          # key chunks per core, 3
SPAN = 256                # keys per q tile: [i-64, i+192)
P = 128

F32 = mybir.dt.float32
BF16 = mybir.dt.bfloat16

NEG = -1e30          # additive mask value
WARMUP_MMS = 8       # dummy matmuls during the DMA ramp to lift the PE HAM
TR_DELAY = 2         # heads between scores and their transposes

# packed xc layout (bf16 columns): x (4 input-dim chunks x HALO) | ident |
# mask0 | mask1 | bq (4 f32 as 8 bf16) | bk
XC_X = 4 * HALO                  # 1536
XC_ID = XC_X                     # 1536
XC_M0 = XC_ID + P                # 1664
XC_M1 = XC_M0 + SPAN             # 1920
XC_BQ = XC_M1 + SPAN             # 2176
XC_BK = XC_BQ + 8                # 2184
XCW = XC_BK + 8                  # 2192


def _build_kernel(nc: bass.Bass, reps: int = 1):
    """Emit the SPMD per-core program. All per-core variation comes from the
    input tensors. reps>1 repeats the body inside one NEFF (benchmarking)."""
    act_f = mybir.ActivationFunctionType

    # ---- I/O ----
    xc = nc.dram_tensor("xc", [P, XCW], BF16, kind="ExternalInput").ap()
    wqk = nc.dram_tensor("wqk", [P, 4, 2 * E], BF16, kind="ExternalInput").ap()
    wvo = nc.dram_tensor("wvo", [P, 4, 2 * E], BF16, kind="ExternalInput").ap()
    yT = nc.dram_tensor("yT", [E, SLOC], BF16, kind="ExternalOutput").ap()

    with tile.TileContext(nc) as tc:
        with (
            tc.tile_pool(name="persist", bufs=1) as persist,
            tc.tile_pool(name="work", bufs=12) as work,
            tc.tile_pool(name="ps_qkv", bufs=3, space="PSUM") as ps_qkv,
            tc.tile_pool(name="ps_ptav", bufs=3, space="PSUM") as ps_ptav,
            tc.tile_pool(name="ps_y", bufs=1, space="PSUM") as ps_y,
        ):
            def emit():
                # warm the PE clock gate during the load ramp: dummy
                # matmuls on a zeroed scratch tile, no data dependencies
                if WARMUP_MMS:
                    wsc = work.tile([P, E], BF16, tag="warm", name="warm")
                    nc.gpsimd.memset(wsc[:], 0.0)
                    wps = ps_qkv.tile([P, E], F32, tag="qkv", name="qkv")
                    for _ in range(WARMUP_MMS):
                        nc.tensor.matmul(wps[:], wsc[:, 0:P], wsc[:],
                                         start=True, stop=True)

                # ---- loads, split fine-grained in consumption order:
                # x | wq/wk c0 | consts | wq/wk c1 | wv | wq/wk c2-3 | wo ----
                xc_sb = persist.tile([P, XCW], BF16, tag="xc", name="xc")
                wqk_sb = persist.tile([P, 4, 2 * E], BF16, tag="wqk",
                                      name="wqk")
                wvo_sb = persist.tile([P, 4, 2 * E], BF16, tag="wvo",
                                      name="wvo")
                nc.sync.dma_start(xc_sb[:, 0:XC_X], xc[:, 0:XC_X])
                nc.sync.dma_start(wqk_sb[:, :, 0:SPAN], wqk[:, :, 0:SPAN])
                nc.sync.dma_start(xc_sb[:, XC_X:], xc[:, XC_X:])
                nc.sync.dma_start(wqk_sb[:, :, SPAN:2 * SPAN],
                                  wqk[:, :, SPAN:2 * SPAN])
                nc.sync.dma_start(wvo_sb[:, :, 0:E], wvo[:, :, 0:E])
                nc.sync.dma_start(wqk_sb[:, :, 2 * SPAN:],
                                  wqk[:, :, 2 * SPAN:])
                nc.sync.dma_start(wvo_sb[:, :, E:2 * E], wvo[:, :, E:2 * E])

                x_sb = xc_sb[:, 0:XC_X].rearrange("p (k s) -> p k s", s=HALO)
                ident = xc_sb[:, XC_ID:XC_ID + P]
                mask_sb = [xc_sb[:, XC_M0:XC_M0 + SPAN],
                           xc_sb[:, XC_M1:XC_M1 + SPAN]]
                bq_sb = xc_sb[:, XC_BQ:XC_BQ + 8].bitcast(F32)
                bk_sb = xc_sb[:, XC_BK:XC_BK + 8].bitcast(F32)
                # col c of head-interleaved wq is wqk[:, :, c*256:c*256+128]
                wq_sb = [wqk_sb[:, :, c * SPAN:c * SPAN + P] for c in range(4)]
                wk_sb = [wqk_sb[:, :, c * SPAN + P:(c + 1) * SPAN]
                         for c in range(4)]
                wv_sb = [wvo_sb[:, k, 0:E] for k in range(4)]
                wo_sb = [wvo_sb[:, k, E:2 * E] for k in range(4)]

                # ---- persistent intermediates ----
                qT_sb, kT_sb = [None] * 4, [None] * 4
                # per-head transposed attention: 4 live blocks
                # [t0kc0 | t0kc1 | t1kc1 | t1kc2], each [128k x 128q]
                attbuf = [persist.tile([P, 4 * P], BF16, tag=f"attT{h}",
                                       name=f"attT{h}") for h in range(H)]
                valsT_sb = [persist.tile([P, SLOC], BF16, tag=f"valsT{c}",
                                         name=f"valsT{c}") for c in range(4)]
                # y accumulators: o-chunks packed in pairs per PSUM bank
                y_bank = [ps_y.tile([P, 2 * SLOC], F32, tag=f"y{i}",
                                    name=f"y{i}") for i in range(2)]
                y_ps = [y_bank[o // 2][:, (o % 2) * SLOC:(o % 2 + 1) * SLOC]
                        for o in range(4)]

                scale = 1.0 / float(np.sqrt(HD))
                att_of = [None] * H     # per-head normalized att [P, 2*SPAN]
                sc_of = [None] * H      # per-head scores PSUM [P, 2*SPAN]

                def emit_qk_chunk(c):
                    for nm, dst, w_sb, b_sb, cols in (
                        ("q", qT_sb, wq_sb, bq_sb, SLOC),
                        ("k", kT_sb, wk_sb, bk_sb, HALO),
                    ):
                        x_off = HWIN if cols == SLOC else 0
                        ps = ps_qkv.tile([P, E], F32, tag="qkv", name="qkv")
                        for k in range(4):
                            nc.tensor.matmul(
                                ps[:, 0:cols], w_sb[c][:, k, :],
                                x_sb[:, k, x_off:x_off + cols],
                                start=(k == 0), stop=(k == 3),
                            )
                        sb = persist.tile([P, cols], BF16, tag=f"{nm}T{c}",
                                          name=f"{nm}T{c}")
                        nc.vector.tensor_scalar_add(
                            sb[:], ps[:, 0:cols], b_sb[:, c:c + 1])
                        dst[c] = sb

                def emit_v_proj():
                    v_sb = []
                    for skc in range(KC):
                        ps = ps_ptav.tile([P, E], F32, tag="ptav",
                                          name="v")
                        for k in range(4):
                            nc.tensor.matmul(
                                ps[:], x_sb[:, k, skc * P:(skc + 1) * P],
                                wv_sb[k], start=(k == 0), stop=(k == 3),
                            )
                        sb = persist.tile([P, E], BF16, tag=f"v{skc}",
                                          name=f"v{skc}")
                        if skc == 1:
                            nc.scalar.copy(sb[:], ps[:])
                        else:
                            nc.vector.tensor_copy(sb[:], ps[:])
                        v_sb.append(sb)
                    return v_sb

                def emit_scores(h):
                    """Both q-tiles of head h into one PSUM bank, then
                    exp + row sums + normalize (DVE) per tile."""
                    c, r = h // 2, (h % 2) * HD
                    scores = ps_qkv.tile([P, 2 * SPAN], F32, tag="qkv",
                                         name="sc")
                    sc_of[h] = scores
                    for t in range(NT):
                        sl = scores[:, t * SPAN:(t + 1) * SPAN]
                        # mask first: it only needs early-arriving consts,
                        # so the in-order PE starts the group before qT/kT
                        nc.tensor.matmul(sl, ident, mask_sb[t],
                                         start=True, stop=False)
                        nc.tensor.matmul(
                            sl,
                            qT_sb[c][r:r + HD, t * P:(t + 1) * P],
                            kT_sb[c][r:r + HD, t * P:t * P + SPAN],
                            start=False, stop=True,
                        )

                def emit_softmax(h):
                    scores = sc_of[h]
                    att = work.tile([P, 2 * SPAN], BF16, tag="att",
                                    name="att")
                    att_of[h] = att
                    for t in range(NT):
                        sl = scores[:, t * SPAN:(t + 1) * SPAN]
                        p = work.tile([P, SPAN], F32, tag="p", name="p")
                        sums = work.tile([P, 1], F32, tag="sums",
                                         name="sums")
                        nc.scalar.activation(p[:], sl, act_f.Exp,
                                             scale=scale, accum_out=sums[:])
                        recip = work.tile([P, 1], F32, tag="recip",
                                          name="recip")
                        nc.vector.reciprocal(recip[:], sums[:])
                        att_sl = att[:, t * SPAN:(t + 1) * SPAN]
                        # t1 normalize on Pool (SBUF->SBUF is legal there):
                        # its transpose is TR_DELAY heads away, so the slow
                        # engine is off the critical path
                        if t == 0 or h == H - 1:
                            nc.vector.tensor_scalar_mul(att_sl, p[:],
                                                        recip[:])
                        else:
                            nc.gpsimd.tensor_scalar_mul(att_sl, p[:],
                                                        recip[:])

                def emit_transpose(h):
                    """4 PE transposes into one [P,512] PSUM pair in AV
                    block order, then a single copy to the head's buffer."""
                    att = att_of[h]
                    ptp = ps_ptav.tile([P, 2 * SPAN], BF16, tag="ptav",
                                       name="pt")
                    for b in range(4):
                        nc.tensor.transpose(
                            ptp[:, b * P:(b + 1) * P],
                            att[:, b * P:(b + 1) * P],
                            ident,
                        )
                    nc.vector.tensor_copy(attbuf[h][:], ptp[:])

                def emit_av(h):
                    c, r = h // 2, (h % 2) * HD
                    ab = attbuf[h]
                    av = ps_ptav.tile([HD, SLOC], F32, tag="ptav",
                                      name="av")
                    # block layout [t0kc0|t0kc1|t1kc1|t1kc2]; q-tile t sums
                    # key chunks t and t+1
                    for t in range(NT):
                        for j, kc in enumerate((t, t + 1)):
                            nc.tensor.matmul(
                                av[:, t * P:(t + 1) * P],
                                v_sb[kc][:, h * HD:(h + 1) * HD],
                                ab[:, (2 * t + j) * P:(2 * t + j + 1) * P],
                                start=(j == 0), stop=(j == 1),
                            )
                    # PSUM source: only DVE/Act may read it (not GPSIMD).
                    # Odd heads feed the y accumulation right away -> fast
                    # DVE; even heads have a head of slack -> Act.
                    if h % 2 == 0:
                        nc.scalar.copy(valsT_sb[c][r:r + HD, :], av[:])
                    else:
                        nc.vector.tensor_copy(valsT_sb[c][r:r + HD, :],
                                              av[:])

                def emit_y_accum(f):
                    # each PSUM bank holds TWO o-regions but may have only
                    # ONE open accumulation group: the even o's start=True
                    # resets the whole bank (zeroing the odd region), and
                    # the odd o accumulates start=False within that group
                    for o in range(4):
                        nc.tensor.matmul(
                            y_ps[o], wo_sb[f][:, o * P:(o + 1) * P],
                            valsT_sb[f][:],
                            start=(f == 0 and o % 2 == 0), stop=(f == 3),
                            skip_group_check=(o % 2 == 1),
                        )

                # ---- main pass: per head, software-pipelined on PE ----
                # PE order per h: AV(h-3) [+y] fills, transposes(h-2), QK
                # chunk (h even), V proj (h==2), scores(h). The softmax of
                # h runs on Act/DVE/Pool while PE is in heads h+1..h+2.
                v_sb = None
                for h in range(H):
                    c = h // 2
                    if h >= 3:
                        emit_av(h - 3)
                        if (h - 3) % 2 == 1:
                            emit_y_accum((h - 3) // 2)
                    if h % 2 == 0:
                        emit_qk_chunk(c)
                    if h == 2:
                        v_sb = emit_v_proj()
                    # transposes after the QK matmuls: they are ready (their
                    # norm ran TR_DELAY heads ago) and hide the bias-add
                    # latency the first scores matmul waits on
                    if h >= TR_DELAY:
                        emit_transpose(h - TR_DELAY)
                    emit_scores(h)
                    emit_softmax(h)
                # epilogue: remaining transposes/AVs interleaved so the
                # PE keeps filling while the last softmax chains drain
                emit_transpose(6)
                emit_av(5)
                emit_y_accum(2)
                emit_transpose(7)
                emit_av(6)
                emit_av(7)
                emit_y_accum(3)

                # ---- output: two bank-wide copies on different engines,
                # two DMAs issued from different queues in parallel ----
                ysb = work.tile([P, 4 * SLOC], BF16, tag="yt", name="yt")
                y4 = yT.rearrange("(o p) q -> p o q", p=P)
                ysb3 = ysb[:].rearrange("p (o q) -> p o q", q=SLOC)
                nc.vector.tensor_copy(ysb[:, 0:2 * SLOC], y_bank[0][:])
                nc.sync.dma_start(y4[:, 0:2, :], ysb3[:, 0:2, :])
                nc.scalar.copy(ysb[:, 2 * SLOC:], y_bank[1][:])
                nc.scalar.dma_start(y4[:, 2:4, :], ysb3[:, 2:4, :])

            for _rep in range(reps):
                emit()

    return nc


_prog_cache = {}


def _get_program(reps: int = 1):
    key = (WARMUP_MMS, reps)
    if key not in _prog_cache:
        nc = bacc.Bacc(
            "TRN2", target_bir_lowering=False, debug=False,
            num_devices=N_CORES,
        )
        _build_kernel(nc, reps=reps)
        nc.compile()
        _prog_cache[key] = nc
    return _prog_cache[key]


def _make_in_maps(x, w_qkv, b_qkv, w_o):
    import ml_dtypes
    bf16 = ml_dtypes.bfloat16

    x2 = np.ascontiguousarray(np.asarray(x, np.float32).reshape(S, E))
    w_qkv = np.asarray(w_qkv, np.float32)
    b_qkv = np.asarray(b_qkv, np.float32)
    w_o = np.asarray(w_o, np.float32)

    # w_qkv rows for head h: [h*3hd, h*3hd+hd) = q, +hd = k, +2hd = v
    idx_q = np.concatenate(
        [np.arange(h * 3 * HD, h * 3 * HD + HD) for h in range(H)])
    idx_k = idx_q + HD
    idx_v = idx_q + 2 * HD
    wqT = w_qkv[idx_q].T.astype(bf16)   # [in, (h,d)]
    wkT = w_qkv[idx_k].T.astype(bf16)
    wvT = w_qkv[idx_v].T.astype(bf16)
    woT = w_o.T.astype(bf16)            # [(h,d), out]
    # f32 biases bit-packed into pairs of bf16 columns
    bq = np.ascontiguousarray(
        b_qkv[idx_q].reshape(4, P).T).view(bf16)          # [p, 8]
    bk = np.ascontiguousarray(b_qkv[idx_k].reshape(4, P).T).view(bf16)
    ident = np.eye(P, dtype=bf16)

    # wqk packed [p, k, c*256 + (wq_c | wk_c)]
    wqk = np.zeros((P, 4, 2 * E), bf16)
    wvo = np.zeros((P, 4, 2 * E), bf16)
    for k in range(4):
        for c in range(4):
            wqk[:, k, c * SPAN:c * SPAN + P] = \
                wqT[k * P:(k + 1) * P, c * P:(c + 1) * P]
            wqk[:, k, c * SPAN + P:(c + 1) * SPAN] = \
                wkT[k * P:(k + 1) * P, c * P:(c + 1) * P]
        wvo[:, k, 0:E] = wvT[k * P:(k + 1) * P, :]
        wvo[:, k, E:2 * E] = woT[k * P:(k + 1) * P, :]

    xT = x2.T  # [E, S] f32
    in_maps = []
    for core in range(N_CORES):
        q0 = core * SLOC
        lo = q0 - HWIN
        xt = np.zeros((E, HALO), np.float32)
        slo, shi = max(lo, 0), min(q0 + SLOC + HWIN, S)
        xt[:, slo - lo: shi - lo] = xT[:, slo:shi]

        m = np.full((NT, P, SPAN), NEG, np.float32)
        for t in range(NT):
            # key position for span col j: q0 + t*128 - 64 + j
            kpos = q0 + t * P - HWIN + np.arange(SPAN)
            qpos = (q0 + t * P + np.arange(P))[:, None]
            valid = (np.abs(kpos[None, :] - qpos) <= HWIN) \
                & (kpos[None, :] >= 0) & (kpos[None, :] < S)
            m[t] = np.where(valid, 0.0, NEG)

        xcore = np.empty((P, XCW), bf16)
        # x chunks: xcore[p, k*HALO + s] = xT[k*128+p, s]
        xcore[:, 0:XC_X] = xt.reshape(4, P, HALO).transpose(1, 0, 2) \
            .reshape(P, XC_X).astype(bf16)
        xcore[:, XC_ID:XC_ID + P] = ident
        xcore[:, XC_M0:XC_M0 + SPAN] = m[0].astype(bf16)
        xcore[:, XC_M1:XC_M1 + SPAN] = m[1].astype(bf16)
        xcore[:, XC_BQ:XC_BQ + 8] = bq
        xcore[:, XC_BK:XC_BK + 8] = bk
        in_maps.append({
            "xc": np.ascontiguousarray(xcore),
            "wqk": wqk, "wvo": wvo,
        })
    return in_maps


last_result = None  # BassKernelResults of the most recent run (for profiling)


def kernel(x, padding_mask, w_qkv, b_qkv, w_o, b_o, trace=False):
    global last_result
    b_qkv = np.asarray(b_qkv, np.float32)
    w_o = np.asarray(w_o, np.float32)
    b_o = np.asarray(b_o, np.float32)
    idx_v = np.concatenate(
        [np.arange(h * 3 * HD + 2 * HD, (h + 1) * 3 * HD) for h in range(H)])
    # rows of softmax sum to 1 (padding_mask is all ones per spec), so the
    # v/out biases commute to a constant output offset; b_qkv[idx_v] is in
    # (h,d) order, matching w_o's input order
    b_eff = b_o + w_o @ b_qkv[idx_v]

    nc = _get_program()
    in_maps = _make_in_maps(x, w_qkv, b_qkv, w_o)
    res = run_bass_kernel_spmd(
        nc, in_maps, core_ids=list(range(N_CORES)), trace=trace)
    last_result = res
    y = np.concatenate(
        [r["yT"].astype(np.float32).T for r in res.results], axis=0)  # [S, E]
    y = y + b_eff[None, :]
    return y.reshape(1, S, E).astype(np.float32)
